# revision 9
# baseline (speedup 1.0000x reference)
"""Trainium2 Bass kernel for nn_Lowpass: 2D DCT -> keep 15x15 low-freq block -> 2D IDCT.

The whole op collapses to out[b,c] = P @ x[b,c] @ P^T with P = Di[:, :15] @ D[:15, :]
(a fixed 32x32 projection). v2 design: both HBM DMAs are fully contiguous
(4 KB per partition line -> full DMA-bus rate; the baseline's (r,h)-partition
gather used 128 B descriptors = half rate and was DMA-bound). The three
cross-partition shuffles that contiguity displaces run on-chip as DVE 32x32
stream transposes (pre/mid/post, provably minimal), which also serve as the
PSUM evictions. Matmuls are full-array K=M=128 with a constant
block-diag(P^T) lhsT in float32r (1 PE cycle/row vs fp32's 4).

Per 128-image pack (1 image per partition, [128, 1024] tiles, 0.5 MB):
  load  L[32a+q, 32h+w] = X[h,w]            (contiguous DMA, sync engine)
  T_pre  T1[32a+h, 32w+q] = X[h,w]           (DVE, src enum "(w h)")
  MM1    P1[32a+v, 32w+q] = (P X)[v,w]       (2 matmuls -> 2 PSUM banks)
  T_mid  T2[32a+w, 32q+v] = (P X)[v,w]       (DVE, src enum "(q w)", evicts P1)
  MM2    P2[32a+u, 32q+v] = (P X P^T)[v,u]   (2 matmuls -> 2 PSUM banks)
  T_post S[32a+q, 32v+u]  = Y[v,u]           (DVE, src enum "(v q)", evicts P2)
  store  contiguous DMA (scalar engine)
PSUM: 2 rounds x 2 banks x 2-deep pools = all 8 banks, fully double buffered.
Data parallel across 8 NeuronCores: 3072 images per core.
"""

import numpy as np

N = 32
FRE = 15
NCORES = 8
IMG_TOTAL = 8192 * 3          # 24576 images of 32x32
PER_CORE = IMG_TOTAL // NCORES  # 3072
PACK = 128                    # images per pipeline iteration (0.5 MB)
NPACK = PER_CORE // PACK      # 24


def _install_tilefix():
    """This container's walrus build rejects instructions carrying >1 sem wait
    ("Too many sync wait commands" in setupSyncWait). Tile attaches all of an
    instruction's required waits to the instruction itself. Split: for any
    instruction with N>1 waits, hoist N-1 of them onto fresh same-engine nop
    instructions placed immediately before it (same blocking semantics, one
    wait per instruction). Same treatment for the kernel-tail drain."""
    from concourse import mybir, tile
    from concourse.vector_clock import ScopedClock, VectorClock

    if getattr(tile.TileContext, "_tilefix_installed", False):
        return

    orig_lower = tile.TileContext._lower_ordered_insts

    def _lower_split(self, postordered_blocks):
        nc = self.nc
        for insts in postordered_blocks.values():
            new = []
            for inst in insts:
                si = getattr(inst, "sync_info", None)
                ow = list(si.on_wait) if si is not None and si.on_wait else []
                if len(ow) > 1:
                    for w in ow[:-1]:
                        nop = mybir.InstNoOp(
                            name=nc.get_next_instruction_name(), ins=[], outs=[])
                        nop.engine = inst.engine
                        nop.sync_info = mybir.SyncInfo(
                            on_wait=[w], on_update=[])
                        new.append(nop)
                    inst.sync_info = mybir.SyncInfo(
                        on_wait=[ow[-1]], on_update=list(si.on_update))
                new.append(inst)
            insts[:] = new
        return orig_lower(self, postordered_blocks)

    def _drain_and_barrier_split(self, tick_clock, wait_clock):
        nc = self.nc
        gc = tick_clock.global_clock
        n = len(gc)
        for proc in range(n):
            t = gc[proc]
            if t <= 0:
                continue
            vec = [0] * n
            vec[proc] = t
            nop_inst = nc.sync.nop()
            wait_clock.add_sem_waits(
                nop_inst.ins, ScopedClock({None: VectorClock(vec)})
            )
        nc.sync.drain()
        nc.all_engine_barrier()
        assert self.sems is not None
        popped = nc._tile_sem_poison_stack.pop()
        assert popped is self._sem_poison
        nc.clear_and_free_semaphores(list(self.sems.allocated().values()))
        nc.all_engine_barrier()

    tile.TileContext._lower_ordered_insts = _lower_split
    tile.TileContext._drain_and_barrier = _drain_and_barrier_split
    tile.TileContext._tilefix_installed = True

    # NTFF profiling hooks don't exist in this container; make trace=True
    # degrade gracefully inside run_bass_kernel_spmd.
    import sys as _sys
    import types as _types
    if "antenv.axon_hooks" not in _sys.modules:
        m = _types.ModuleType("antenv.axon_hooks")
        m.get_axon_ntff_profile_hook = lambda: None
        _sys.modules["antenv.axon_hooks"] = m


def _p_matrix():
    i = np.arange(N)
    D = 2.0 * np.cos(np.pi * (2 * i[None, :] + 1) * i[:, None] / (2 * N))
    Di = np.linalg.inv(D)
    P = Di[:, :FRE] @ D[:FRE, :]        # float64 [32, 32]
    return P


def _bd_matrix():
    # lhsT = block-diag(P^T): BD[32A+h, 32A+v] = P[v, h]
    P = _p_matrix()
    return np.kron(np.eye(4), P.T).astype(np.float32)  # [128, 128]


def _build_program(mm_dtype_name="float32r", loop_reps=1, dma_only=False):
    from concourse import bass, tile
    from concourse import mybir

    F32 = mybir.dt.float32
    MMDT = getattr(mybir.dt, mm_dtype_name)
    FREE = PACK * 8             # 1024 free elems per [128, FREE] tile

    nc = bass.Bass("TRN2", target_bir_lowering=False, debug=False,
                   num_devices=NCORES)
    x_ext = nc.dram_tensor("x", [PER_CORE, N, N], F32, kind="ExternalInput").ap()
    p_ext = nc.dram_tensor("pconst", [128, 128], F32, kind="ExternalInput").ap()
    y_ext = nc.dram_tensor("y", [PER_CORE, N, N], F32, kind="ExternalOutput").ap()

    need_round = mm_dtype_name != "float32"

    with tile.TileContext(nc) as tc:
        with tc.tile_pool(name="const", bufs=1) as cpool, \
             tc.tile_pool(name="xin", bufs=3) as xpool, \
             tc.tile_pool(name="t1", bufs=2) as t1pool, \
             tc.tile_pool(name="t1r", bufs=2) as t1rpool, \
             tc.tile_pool(name="t2", bufs=2) as t2pool, \
             tc.tile_pool(name="t2r", bufs=2) as t2rpool, \
             tc.tile_pool(name="sout", bufs=2) as spool, \
             tc.tile_pool(name="psA", bufs=2, space="PSUM") as papool, \
             tc.tile_pool(name="psB", bufs=2, space="PSUM") as pbpool:

            # fp32r matmul operands must be produced by an instruction that
            # rounds to fp32r (walrus checkMatmultFP32r). DMA and
            # StreamTranspose don't qualify (the latter rejects fp32r/dtype
            # conversion outright), so the BD constant and both transposed
            # operands get an ACT convert-copy in front of the matmuls.
            if need_round:
                bd_stage = cpool.tile([128, 128], F32)
                nc.sync.dma_start(bd_stage[:], p_ext[:])
                bd_mm = cpool.tile([128, 128], MMDT)
                nc.scalar.copy(bd_mm[:], bd_stage[:])
            else:
                bd_mm = cpool.tile([128, 128], F32)
                nc.sync.dma_start(bd_mm[:], p_ext[:])

            for p_rep in range(NPACK * loop_reps):
                p = p_rep % NPACK
                base = p * PACK
                # ---- load: ONE contiguous 0.5MB DMA (4KB/partition) ----
                L = xpool.tile([128, FREE], F32)
                nc.sync.dma_start(
                    L.rearrange("p (h w) -> p h w", w=N),
                    x_ext[base: base + PACK],
                )

                if dma_only:
                    nc.scalar.dma_start(
                        y_ext[base: base + PACK],
                        L.rearrange("p (h w) -> p h w", w=N),
                    )
                    continue

                # ---- T_pre: T1[32a+h, 32w+q] = X[h,w] ----
                T1 = t1pool.tile([128, FREE], F32)
                nc.vector.transpose(
                    T1.rearrange("p (w q) -> p w q", q=N),
                    L.rearrange("p (h w) -> p w h", w=N),
                )
                if need_round:
                    T1_mm = t1rpool.tile([128, FREE], MMDT)
                    nc.scalar.copy(T1_mm[:], T1[:])
                else:
                    T1_mm = T1

                # ---- MM1: P1[32a+v, 32w+q] = (P X)[v,w] ----
                P1 = papool.tile([128, FREE], F32, tag="psA")
                for b in range(2):
                    nc.tensor.matmul(
                        P1[:, 512 * b: 512 * (b + 1)],
                        bd_mm[:, :],
                        T1_mm[:, 512 * b: 512 * (b + 1)],
                        start=True, stop=True,
                    )

                # ---- T_mid: T2[32a+w, 32q+v] = (P X)[v,w] (evicts P1) ----
                T2 = t2pool.tile([128, FREE], F32)
                nc.vector.transpose(
                    T2.rearrange("p (q v) -> p q v", v=N),
                    P1.rearrange("p (w q) -> p q w", q=N),
                )
                if need_round:
                    T2_mm = t2rpool.tile([128, FREE], MMDT)
                    nc.scalar.copy(T2_mm[:], T2[:])
                else:
                    T2_mm = T2

                # ---- MM2: P2[32a+u, 32q+v] = Y[v,u] ----
                P2 = pbpool.tile([128, FREE], F32, tag="psB")
                for b in range(2):
                    nc.tensor.matmul(
                        P2[:, 512 * b: 512 * (b + 1)],
                        bd_mm[:, :],
                        T2_mm[:, 512 * b: 512 * (b + 1)],
                        start=True, stop=True,
                    )

                # ---- T_post: S[32a+q, 32v+u] = Y[v,u] (evicts P2) ----
                S = spool.tile([128, FREE], F32)
                nc.vector.transpose(
                    S.rearrange("p (v u) -> p v u", u=N),
                    P2.rearrange("p (q v) -> p v q", v=N),
                )

                # ---- store: ONE contiguous 0.5MB DMA ----
                nc.scalar.dma_start(
                    y_ext[base: base + PACK],
                    S.rearrange("p (h w) -> p h w", w=N),
                )

    return nc


def _run(x_flat, trace=False, mm_dtype_name="float32r"):
    from concourse.bass_utils import run_bass_kernel_spmd

    _install_tilefix()
    nc = _build_program(mm_dtype_name)

    pconst = _bd_matrix()

    core_ids = list(range(NCORES))
    in_maps = [
        {"x": np.ascontiguousarray(x_flat[i * PER_CORE:(i + 1) * PER_CORE]),
         "pconst": pconst}
        for i in core_ids
    ]
    bkr = run_bass_kernel_spmd(nc, in_maps, core_ids, trace=trace)
    out = np.concatenate([bkr.results[i]["y"] for i in core_ids], axis=0)
    return out, bkr


def kernel(x):
    x = np.asarray(x, dtype=np.float32)
    x_flat = x.reshape(IMG_TOTAL, N, N)
    out, _ = _run(x_flat, trace=False)
    return out.reshape(x.shape).astype(np.float32)
